# revision 12
# baseline (speedup 1.0000x reference)
"""BitNet ternary linear (nn_BitNetLinear4Bit) Trainium2 Bass kernel.

out = x @ (alpha * clip(round(w/alpha), -1, 1))^T + bias
  x: [2, 2048, 4096] f32, w: [11008, 4096] f32, alpha: [1] f32, bias: [11008] f32
  -> out: [2, 2048, 11008] f32

Sharding: column-parallel over 8 cores. Each core gets the full x
(replicated) and a 1376-row slice of w / bias; it produces a
[4096, 1376] slice of the output which the host concatenates.

Per-core algorithm (all math on device). HW measurements that shaped
it (from perfetto traces of earlier versions):
  - matmul issue gap is N/2.4GHz + 2.5ns regardless of dtype or
    perf_mode; an fp8e4 DoubleRow matmul covers TWO 128-deep k-tiles
    per instruction at the same N-cost => 2x throughput per k-tile.
  - e4m3 quantization of x costs 2.35e-2 rel err if applied to all of
    K; applied to k<2560 it contributes sqrt(2560/4096)*2.35e-2 and
    the bf16 rest is exact-ish => total 1.87e-2 < 2e-2 gate (verified
    numerically on the fixed-seed inputs; ternary weights are EXACT
    in fp8/bf16 so they add no error; HW matched the numpy sim to
    1e-4 on several runs).
  - ALL XBAR transposes must stay on ONE HWDGE queue: issuing them
    concurrently from sync and scalar queues corrupted results.
  - engine queues are strict FIFO; a semaphore wait at the head
    blocks everything behind it. Queue assignment: Pool/SWDGE does
    x casting loads + w f32 loads; sync does XBAR transposes only;
    scalar does ACT Sign + output stores; DVE does ternarize
    compares, fp8 casts, and evictions — with x tiles prefetched TWO
    token-blocks ahead in emission order so their casts sit ahead of
    evictions (whose sems resolve late) in the DVE FIFO.
  - weight groups are (2,2,3,4) chunks wide and phase W is emitted
    interleaved with the first PREFIX_MS token blocks per group, so
    the PE starts ~25us in and weight prep hides under matmuls.

Per 128-token block, per output group (256/256/384/480 cols):
10 DoubleRow fp8 matmuls (k<2560, x in e4m3) + 12 bf16 matmuls
accumulate K=4096 into one PSUM bank; one DVE scalar_tensor_tensor
evicts psum*(alpha/2) + bias (weights are stored as 2t in {-2,0,2});
scalar-queue DMA stores. Ternarize: even chunks use ACT Sign
(2t = sign(w-a/2) + sign(w+a/2), Sign is a native 1-bucket exact op),
odd chunks use DVE compares (2t = 2*(w>=a/2) - 2*(w<=-a/2)) so the
two engines pace the weight phase in parallel.

alpha is read on the host and baked into the program as an immediate;
the compiled program is cached keyed on alpha and recompiled if it
changes.
"""

import numpy as np

B, S, DIN, DOUT = 2, 2048, 4096, 11008
NCORES = 8
DOUT_SH = DOUT // NCORES  # 1376
TOK = B * S  # 4096
P = 128

KF8 = 2560  # k range [0, KF8) in fp8 DoubleRow pairs; mult of 256
KOF = KF8 // P  # 20 fp8 ko levels
KPAIR = KOF // 2  # 10 DoubleRow matmuls per group per token block
KOB = DIN // P - KOF  # 12 bf16 ko levels
PREFIX_MS = 5  # token blocks emitted group-major for W/MM overlap
GROUP_CC = (2, 2, 3, 4)  # chunks per output group (first groups small)


def _build(alpha_f, TOK=TOK, DIN=DIN, DOUT_SH=DOUT_SH, debug=False):
    import concourse.mybir as mybir
    from concourse import bacc
    from concourse.tile import TileContext

    f32 = mybir.dt.float32
    bf16 = mybir.dt.bfloat16
    f8 = mybir.dt.float8e4
    Alu = mybir.AluOpType
    Act = mybir.ActivationFunctionType
    DR = mybir.MatmulPerfMode.DoubleRow

    KO = DIN // P  # 32
    M_SUBS = TOK // P  # 32
    W_CHUNKS = (DOUT_SH + P - 1) // P  # 11 (last chunk 96 rows, zero-padded)
    HCOL = 2048  # w rows stream in two 2048-col halves
    GROUPS = []  # (first chunk, n chunks, dout start, real width)
    c = 0
    for cc in GROUP_CC:
        width = min(DOUT_SH, (c + cc) * P) - c * P
        GROUPS.append((c, cc, c * P, width))
        c += cc
    assert c == W_CHUNKS

    a2 = float(alpha_f) * 0.5

    nc = bacc.Bacc(None, target_bir_lowering=False, debug=debug)
    x_d = nc.dram_tensor("x", [TOK, DIN], f32, kind="ExternalInput")
    w_d = nc.dram_tensor("w", [DOUT_SH, DIN], f32, kind="ExternalInput")
    nc.dram_tensor("alpha", [1], f32, kind="ExternalInput")
    b_d = nc.dram_tensor("bias", [DOUT_SH], f32, kind="ExternalInput")
    o_d = nc.dram_tensor("out", [TOK, DOUT_SH], f32, kind="ExternalOutput")

    with TileContext(nc) as tc:
        with (
            tc.tile_pool(name="const", bufs=1) as const,
            tc.tile_pool(name="wres", bufs=1) as wres,
            tc.tile_pool(name="wq", bufs=2) as wq,
            tc.tile_pool(name="wtt", bufs=1) as wtt,
            tc.tile_pool(name="xp", bufs=2) as xp,
            tc.tile_pool(name="xtp", bufs=3) as xtp,
            tc.tile_pool(name="xbp", bufs=6) as xbp,
            tc.tile_pool(name="x8p", bufs=6) as x8p,
            tc.tile_pool(name="op", bufs=4) as op,
            tc.tile_pool(name="pso", bufs=8, space="PSUM") as pso,
        ):
            bias_sb = const.tile([P, DOUT_SH], f32)
            nc.sync.dma_start(
                bias_sb[:],
                b_d[:].rearrange("(a n) -> a n", a=1).to_broadcast((P, DOUT_SH)),
            )
            # per-partition scalar biases for the ACT Sign ternarize
            bneg = const.tile([P, 1], f32)
            nc.gpsimd.memset(bneg[:], -a2)
            bpos = const.tile([P, 1], f32)
            nc.gpsimd.memset(bpos[:], a2)

            # resident transposed ternary weights (stored as 2t):
            # fp8:  wtf[g][p, ko, i*128+j] = 2t[(c0+i)*128+j, ko*128+p]
            # bf16: wtb[g][p, i, kb, j]   = 2t[(c0+i)*128+j, (KOF+kb)*128+p]
            wtf = [
                wres.tile([P, KOF, cc * P], f8, name=f"wtf_{g}")
                for g, (_, cc, _, _) in enumerate(GROUPS)
            ]
            wtb = [
                wres.tile([P, cc, KOB, P], bf16, name=f"wtb_{g}")
                for g, (_, cc, _, _) in enumerate(GROUPS)
            ]

            def emit_w_group(g):
                c0, cc, n0, width = GROUPS[g]
                for i in range(cc):
                    c = c0 + i
                    rc = min(P, DOUT_SH - c * P)  # 128 or 96 (last)
                    # ternarize the full 4096-col row in two 2048 halves;
                    # alternate engines per chunk so ACT and DVE pace the
                    # weight phase in parallel. Both produce 2t in {-2,0,2}.
                    tqf = wq.tile([P, DIN], bf16, tag="tqf")
                    for h in range(2):
                        wrow = wq.tile([P, HCOL], f32, tag="wrow")
                        if rc < P:
                            nc.gpsimd.memset(wrow[:], 0.0)
                        # w loads ride the Pool/SWDGE queue (plain f32)
                        nc.gpsimd.dma_start(
                            wrow[:rc, :],
                            w_d[c * P : c * P + rc, h * HCOL : (h + 1) * HCOL],
                        )
                        dst = tqf[:, h * HCOL : (h + 1) * HCOL]
                        if c % 2 == 0:
                            # 2t = sign(w - a/2) + sign(w + a/2) (ACT Sign
                            # is a native 1-bucket op => exact)
                            s1 = wq.tile([P, HCOL], bf16, tag="s1")
                            nc.scalar.activation(
                                s1[:], wrow[:], Act.Sign, bias=bneg[:]
                            )
                            s2 = wq.tile([P, HCOL], bf16, tag="s2")
                            nc.scalar.activation(
                                s2[:], wrow[:], Act.Sign, bias=bpos[:]
                            )
                            nc.vector.tensor_tensor(dst, s1[:], s2[:], Alu.add)
                        else:
                            # 2t = 2*(w >= a/2) - 2*(w <= -a/2) (DVE, f32)
                            s1 = wq.tile([P, HCOL], bf16, tag="s1")
                            nc.vector.tensor_scalar(
                                s1[:], wrow[:], -a2, 2.0, Alu.is_le, Alu.mult
                            )
                            s2 = wq.tile([P, HCOL], bf16, tag="s2")
                            nc.vector.tensor_scalar(
                                s2[:], wrow[:], a2, 2.0, Alu.is_ge, Alu.mult
                            )
                            nc.vector.tensor_tensor(
                                dst, s2[:], s1[:], Alu.subtract
                            )
                    # ONE full-row XBAR transpose (sync queue — all XBAR
                    # transposes must share a single queue), then split:
                    # DVE-cast the fp8 part, DVE-copy the bf16 part.
                    wtT = wtt.tile([P, KO, P], bf16, tag="wtT")
                    nc.sync.dma_start_transpose(wtT[:], tqf[:])
                    nc.vector.tensor_copy(
                        wtf[g][:, :, i * P : (i + 1) * P], wtT[:, :KOF, :]
                    )
                    nc.vector.tensor_copy(wtb[g][:, i, :, :], wtT[:, KOF:, :])

            def emit_x_load(ms):
                # SWDGE casting DMA: f32 HBM -> bf16 SBUF (Pool queue).
                xbf = xp.tile([P, DIN], bf16, tag="xbf", name=f"xbf_{ms}")
                for h in range(2):
                    hw = DIN // 2
                    nc.gpsimd.dma_start(
                        xbf[:, h * hw : (h + 1) * hw],
                        x_d[ms * P : (ms + 1) * P, h * hw : (h + 1) * hw],
                    )
                xt = xtp.tile([P, KO, P], bf16, tag="xt", name=f"xt_{ms}")
                nc.sync.dma_start_transpose(xt[:], xbf[:])
                xt8 = x8p.tile([P, KOF, P], f8, tag="xt8", name=f"xt8_{ms}")
                nc.vector.tensor_copy(xt8[:], xt[:, :KOF, :])
                # compact bf16 copy so xt (8KB/partition) can be recycled
                # while prefix blocks stay live across all group passes
                xtb = xbp.tile([P, KOB, P], bf16, tag="xtb", name=f"xtb_{ms}")
                nc.vector.tensor_copy(xtb[:], xt[:, KOF:, :])
                return xtb, xt8

            def emit_mm(ms, g, xtb, xt8):
                c0, cc, n0, width = GROUPS[g]
                po = pso.tile([P, 512], f32, tag="po", name=f"po_{ms}_{g}")
                pw = cc * P
                for kp in range(KPAIR):
                    nc.tensor.matmul(
                        po[:, :pw],
                        xt8[:, 2 * kp : 2 * kp + 2, :],
                        wtf[g][:, 2 * kp : 2 * kp + 2, :],
                        start=(kp == 0),
                        stop=False,
                        perf_mode=DR,
                    )
                for kb in range(KOB):
                    nc.tensor.matmul(
                        po[:, :pw],
                        xtb[:, kb, :],
                        wtb[g][:, :, kb, :],
                        start=False,
                        stop=(kb == KOB - 1),
                    )
                osb = op.tile([P, 512], f32, tag="osb", name=f"osb_{ms}_{g}")
                # psum holds 2t accumulation => scale by alpha/2
                nc.vector.scalar_tensor_tensor(
                    osb[:, :width],
                    po[:, :width],
                    float(alpha_f) * 0.5,
                    bias_sb[:, n0 : n0 + width],
                    Alu.mult,
                    Alu.add,
                )
                # store on the scalar HWDGE queue: its sem-waits resolve
                # late and would head-block x prefetch (Pool) or x
                # transposes (sync).
                nc.scalar.dma_start(
                    o_d[ms * P : (ms + 1) * P, n0 : n0 + width],
                    osb[:, :width],
                )

            # interleaved emission: W(g) then the first PREFIX_MS token
            # blocks of group g, so strict-FIFO engine queues never
            # head-block the MM pipeline behind the whole W phase.
            x_pre = {}
            x_pre[0] = emit_x_load(0)
            x_pre[1] = emit_x_load(1)
            emit_w_group(0)
            for ms in range(2, PREFIX_MS):
                x_pre[ms] = emit_x_load(ms)
            for g in range(len(GROUPS)):
                if g > 0:
                    emit_w_group(g)
                for ms in range(PREFIX_MS):
                    emit_mm(ms, g, *x_pre[ms])
            # steady state: prefetch x TWO token-blocks ahead in emission
            # order so x casts sit ahead of evictions in the DVE FIFO.
            xq = {}
            for ms in range(PREFIX_MS, min(PREFIX_MS + 2, M_SUBS)):
                xq[ms] = emit_x_load(ms)
            for ms in range(PREFIX_MS, M_SUBS):
                if ms + 2 < M_SUBS:
                    xq[ms + 2] = emit_x_load(ms + 2)
                xtb, xt8 = xq.pop(ms)
                for g in range(len(GROUPS)):
                    emit_mm(ms, g, xtb, xt8)

    nc.compile()
    return nc


_CACHE = {}


def _get_nc(alpha_f):
    key = float(alpha_f)
    if key not in _CACHE:
        _CACHE[key] = _build(key)
    return _CACHE[key]


def kernel(x, w, alpha, bias):
    from concourse.bass_utils import run_bass_kernel_spmd

    alpha2 = np.ascontiguousarray(np.asarray(alpha, dtype=np.float32).reshape(1))
    nc = _get_nc(alpha2[0])
    x2 = np.ascontiguousarray(np.asarray(x, dtype=np.float32).reshape(TOK, DIN))
    in_maps = []
    for c in range(NCORES):
        in_maps.append(
            {
                "x": x2,
                "w": np.ascontiguousarray(w[c * DOUT_SH : (c + 1) * DOUT_SH]),
                "alpha": alpha2,
                "bias": np.ascontiguousarray(bias[c * DOUT_SH : (c + 1) * DOUT_SH]),
            }
        )
    res = run_bass_kernel_spmd(nc, in_maps, core_ids=list(range(NCORES)))
    outs = [res.results[c]["out"] for c in range(NCORES)]
    out = np.concatenate(outs, axis=1).reshape(B, S, DOUT)
    return np.ascontiguousarray(out.astype(np.float32))
